# revision 1
# baseline (speedup 1.0000x reference)
"""DiceLoss kernel for Trainium2 (8 NeuronCores, SPMD data-parallel).

Problem: input [2,4,128,160,160] f32 logits, target [2,128,160,160] int
  pred = argmax(input, axis=1); for classes 1..3 compute
  inter_c = |pred==c & tgt==c|, union_c = |pred==c| + |tgt==c| - inter_c
  loss = 1 - mean_{b,c}( (inter+eps)/(union+eps) )

Sharding: flatten spatial dims (N=3,276,800 voxels per batch) and give each
of the 8 cores a contiguous 1/8 slice (S=409,600 voxels) of BOTH batches.
Each core computes per-(batch, class) partial counts; the host sums the 8
tiny partial-count tensors and finishes the scalar dice math.

Per-core on-chip layout: each (b, class) plane slice is [128 partitions x
3200]; processed in free-dim chunks of 1280/1280/640.

Engine assignment (per chunk):
  DVE : max01/max23/m (f32 max), pm1..3 = is_equal(x_c, m) -> bf16 masks
  ACT : tm_c = Relu(1 - Square(t - c)) -> bf16 one-hot of target, with
        accum_out giving the per-partition target counts for free
  PE  : inter_c via PM_c^T @ TM_c accumulated in PSUM (trace on host),
        pred counts via ones^T @ PM_c column sums
  DMA : PSUM -> DRAM drained directly (no SBUF staging)

(tensor_tensor_reduce and all GpSimd compute fail this toolchain's
walrus codegen — avoid.)

argmax tie semantics: pm_c = (x_c == max). For f32 normal inputs exact ties
have probability ~1e-7 over the whole tensor; the count error is <=O(1) out
of ~1e6, far below tolerance.
"""

import sys

sys.path.insert(0, "/opt/trn_rl_repo")

import numpy as np

# ---------------------------------------------------------------------------
# Hardcoded problem geometry
# ---------------------------------------------------------------------------
B = 2
C = 4
N_SP = 128 * 160 * 160        # 3,276,800 voxels per batch
N_CORES = 8
S = N_SP // N_CORES           # 409,600 voxels per core per batch
P = 128
SF = S // P                   # 3200 free elems per partition
# free-dim chunks (multiples of 128). Sizes ramp up so the first chunk's
# DMA+DVE latency (pipeline fill) is small.
CHUNKS = [(0, 256), (256, 512), (768, 1024), (1792, 1408)]
EPS = 1e-08

_CACHE = {}


def _build_bass(s=S, chunks=None):
    import concourse.bass as bass
    import concourse.tile as tile
    from concourse import bacc, mybir
    from contextlib import ExitStack

    if chunks is None:
        chunks = CHUNKS
    f32 = mybir.dt.float32
    bf16 = mybir.dt.bfloat16
    u8 = mybir.dt.uint8
    Alu = mybir.AluOpType

    # Bacc (not raw Bass): its compile() legalizes sync — multi-wait
    # instructions get their waits split onto event-semaphore nops, which
    # the walrus BIR verifier requires.
    nc = bacc.Bacc()

    n_chunks = len(chunks)
    n_cols = B * n_chunks * 3
    x = nc.declare_dram_parameter("x", [B, C, s], f32, isOutput=False)
    t = nc.declare_dram_parameter("t", [B, s], u8, isOutput=False)
    # acc_out[:, col] = tm_c counts (ACT accum), col = (b*n_chunks+j)*3+(c-1)
    acc_d = nc.declare_dram_parameter("acc_out", [P, n_cols], f32, isOutput=True)
    # diag_out[b][:, (c-1)*128 : c*128] = PM_c^T @ TM_c ; trace = inter_c
    diag_d = nc.declare_dram_parameter("diag_out", [B, P, 384], f32, isOutput=True)
    # cnt_out[b, 0, (c-1)*512:(c)*512] = per-column pm_c counts (PE colsum)
    cnt_d = nc.declare_dram_parameter("cnt_out", [B, 1, 1536], f32, isOutput=True)

    with ExitStack() as ctx:
        tc = ctx.enter_context(tile.TileContext(nc))
        const_pool = ctx.enter_context(tc.tile_pool(name="const", bufs=1))
        xpool = ctx.enter_context(tc.tile_pool(name="xp", bufs=3))
        tpool = ctx.enter_context(tc.tile_pool(name="tp", bufs=3))
        mpool = ctx.enter_context(tc.tile_pool(name="mp", bufs=2))
        kpool = ctx.enter_context(tc.tile_pool(name="kp", bufs=2))
        pspool = ctx.enter_context(tc.tile_pool(name="ps", bufs=1, space="PSUM"))

        acc = const_pool.tile([P, n_cols], f32)
        ones = const_pool.tile([P, 1], bf16)
        nc.vector.memset(ones[:], 1.0)
        # per-partition bias constants for activation (only 0.0/1.0 have
        # builtin const APs)
        neg_c = []
        for c in (1, 2, 3):
            bias_t = const_pool.tile([P, 1], f32, tag=f"bias{c}", name=f"bias{c}")
            nc.vector.memset(bias_t[:], -float(c))
            neg_c.append(bias_t)

        for b in range(B):
            ps_diags = [
                pspool.tile([P, 128], f32, tag=f"diag{ci}", name=f"ps_diag{ci}")
                for ci in range(3)
            ]
            ps_cnts = [
                pspool.tile([1, 512], f32, tag=f"cnt{ci}", name=f"ps_cnt{ci}")
                for ci in range(3)
            ]

            for j, (f0, F) in enumerate(chunks):
                xts = []
                for ci in range(C):
                    xc = xpool.tile([P, F], f32, tag=f"x{ci}", name=f"x{ci}")
                    xsrc = x[b, ci, :].rearrange("(p f) -> p f", p=P)
                    nc.sync.dma_start(out=xc[:], in_=xsrc[:, f0 : f0 + F])
                    xts.append(xc)
                tt = tpool.tile([P, F], u8, tag="tt")
                tsrc = t[b, :].rearrange("(p f) -> p f", p=P)
                nc.sync.dma_start(out=tt[:], in_=tsrc[:, f0 : f0 + F])

                m01 = mpool.tile([P, F], f32, tag="m01")
                nc.vector.tensor_tensor(m01[:], xts[0][:], xts[1][:], op=Alu.max)
                m23 = mpool.tile([P, F], f32, tag="m23")
                nc.vector.tensor_tensor(m23[:], xts[2][:], xts[3][:], op=Alu.max)
                m = mpool.tile([P, F], f32, tag="m")
                nc.vector.tensor_tensor(m[:], m01[:], m23[:], op=Alu.max)

                pms = []
                for ci in range(3):
                    pm = kpool.tile([P, F], bf16, tag=f"pm{ci}", name=f"pm{ci}")
                    nc.vector.tensor_tensor(
                        pm[:], xts[ci + 1][:], m[:], op=Alu.is_equal
                    )
                    pms.append(pm)

                tms = []
                for ci, c in enumerate((1, 2, 3)):
                    sq = kpool.tile([P, F], bf16, tag=f"sq{c}", name=f"sq{c}")
                    nc.scalar.activation(
                        sq[:], tt[:], mybir.ActivationFunctionType.Square,
                        bias=neg_c[ci][:], scale=1.0,
                    )
                    tm = kpool.tile([P, F], bf16, tag=f"tm{c}", name=f"tm{c}")
                    col = (b * n_chunks + j) * 3 + ci
                    nc.scalar.activation(
                        tm[:], sq[:], mybir.ActivationFunctionType.Relu,
                        bias=1.0, scale=-1.0,
                        accum_out=acc[:, col : col + 1],
                    )
                    tms.append(tm)

                last_j = j == n_chunks - 1
                # inter_c: PM_c^T @ TM_c accumulated over the whole batch
                ns128 = F // 128
                for si in range(ns128):
                    sl = slice(si * 128, (si + 1) * 128)
                    for ci in range(3):
                        nc.tensor.matmul(
                            ps_diags[ci][:, :],
                            pms[ci][:, sl],
                            tms[ci][:, sl],
                            start=(j == 0 and si == 0),
                            stop=(last_j and si == ns128 - 1),
                        )
                # pm_c counts: ones^T @ PM_c partition-sums, accumulated
                offs = []
                off = 0
                while off < F:
                    offs.append((off, min(512, F - off)))
                    off += 512
                for ci in range(3):
                    for oi, (off, ns) in enumerate(offs):
                        nc.tensor.matmul(
                            ps_cnts[ci][0:1, 0:ns],
                            ones[:],
                            pms[ci][:, off : off + ns],
                            start=(j == 0 and oi == 0),
                            stop=(last_j and oi == len(offs) - 1),
                        )

            # drain PSUM -> SBUF (DMA cannot read PSUM), then DMA out
            cw = min(512, max(F for _, F in chunks))
            sb_diag = tpool.tile([P, 384], f32, tag="sbd", name="sb_diag")
            for ci in range(3):
                nc.scalar.copy(
                    sb_diag[:, ci * 128 : (ci + 1) * 128], ps_diags[ci][:]
                )
            nc.sync.dma_start(out=diag_d[b, :, :], in_=sb_diag[:])
            sb_cnt = tpool.tile([1, 1536], f32, tag="sbc", name="sb_cnt")
            for ci in range(3):
                nc.vector.tensor_copy(
                    sb_cnt[0:1, ci * 512 : ci * 512 + cw], ps_cnts[ci][0:1, 0:cw]
                )
                # unwritten columns of cnt_d stay zero (outputs are
                # zero-initialized by the runtime)
                nc.sync.dma_start(
                    out=cnt_d[b, :, ci * 512 : ci * 512 + cw],
                    in_=sb_cnt[0:1, ci * 512 : ci * 512 + cw],
                )

        nc.sync.dma_start(out=acc_d[:, :], in_=acc[:])

    nc.compile()
    return nc


def _get_nc():
    if "nc" not in _CACHE:
        _CACHE["nc"] = _build_bass()
    return _CACHE["nc"]


def _shard_inputs(input, target):
    inp = np.ascontiguousarray(input, dtype=np.float32).reshape(B, C, N_SP)
    tgt = np.asarray(target).reshape(B, N_SP)
    in_maps = []
    for r in range(N_CORES):
        xr = np.ascontiguousarray(inp[:, :, r * S : (r + 1) * S])
        tr = np.ascontiguousarray(tgt[:, r * S : (r + 1) * S].astype(np.uint8))
        in_maps.append({"x": xr, "t": tr})
    return in_maps


def _finish(results):
    """Combine per-core partial counts into the dice loss."""
    inter = np.zeros((B, 3), np.float64)
    pred_cnt = np.zeros((B, 3), np.float64)
    tgt_cnt = np.zeros((B, 3), np.float64)
    n_chunks = len(CHUNKS)
    for res in results:
        acc = np.asarray(res["acc_out"], np.float64)        # [128, n_cols]
        diag = np.asarray(res["diag_out"], np.float64)      # [B, 128, 384]
        cnt = np.asarray(res["cnt_out"], np.float64)        # [B, 1, 1536]
        for b in range(B):
            for ci in range(3):
                blk = diag[b][:, ci * 128 : (ci + 1) * 128]
                inter[b, ci] += np.trace(blk)
                pred_cnt[b, ci] += cnt[b, 0, ci * 512 : (ci + 1) * 512].sum()
                for j in range(n_chunks):
                    tgt_cnt[b, ci] += acc[:, (b * n_chunks + j) * 3 + ci].sum()
    union = pred_cnt + tgt_cnt - inter
    dice = (inter + EPS) / (union + EPS)
    return np.float32(1.0 - dice.mean())


def kernel(input, target):
    from concourse.bass_utils import run_bass_kernel_spmd

    nc = _get_nc()
    in_maps = _shard_inputs(input, target)
    out = run_bass_kernel_spmd(nc, in_maps, core_ids=list(range(N_CORES)))
    return _finish(out.results)


if __name__ == "__main__":
    # Smoke test with random data against a numpy reference.
    rng = np.random.default_rng(0)
    inp = rng.standard_normal((B, C, 128, 160, 160), dtype=np.float32)
    tgt = rng.integers(0, C, size=(B, 128, 160, 160)).astype(np.int32)

    got = kernel(input=inp, target=tgt)

    pred = np.argmax(inp, axis=1).reshape(B, -1)
    tg = tgt.reshape(B, -1)
    dice = np.zeros((B, 3))
    for b in range(B):
        for ci, c in enumerate((1, 2, 3)):
            pm = pred[b] == c
            tm = tg[b] == c
            i = np.sum(pm & tm)
            u = np.sum(pm | tm)
            dice[b, ci] = (i + EPS) / (u + EPS)
    want = np.float32(1.0 - dice.mean())
    print("kernel:", got, "reference:", want, "relerr:", abs(got - want) / abs(want))



# revision 2
# speedup vs baseline: 8.9069x; 8.9069x over previous
"""DiceLoss kernel for Trainium2 (8 NeuronCores, SPMD data-parallel).

Problem: input [2,4,128,160,160] f32 logits, target [2,128,160,160] int
  pred = argmax(input, axis=1); for classes 1..3 compute
  inter_c = |pred==c & tgt==c|, union_c = |pred==c| + |tgt==c| - inter_c
  loss = 1 - mean_{b,c}( (inter+eps)/(union+eps) )

Sharding: flatten spatial dims (N=3,276,800 voxels per batch); each of the
8 cores takes a contiguous 1/8 slice (S=409,600 voxels) of BOTH batches.
Each core computes per-(batch, class) partial counts; the host sums the 8
tiny partial-count tensors and finishes the scalar dice math.

v2 design (from NTFF profile of v1: DVE 50us busy on f32 tensor_tensor at
1x mode, DMA issue-bound on one queue, PE burdened by separate count
matmuls):
  - Host converts logits f32 -> bf16 (verified loss rel-err 1.8e-4 vs the
    2e-2 gate; bf16 ties are ~0.2% of voxels and cancel in the ratio).
    Halves HBM traffic and doubles DVE tensor_tensor throughput (2x_1P).
  - Chunks cover contiguous flat ranges reshaped [128, F] so every DMA is
    one contiguous block (3+KB per-partition rows, best DMA-engine rate).
    Counts are permutation-invariant so the reshape order is free.
  - DMA issue split across both HWDGE queues (sync + scalar engines).
  - tm tiles interleave a ones-pair after every 128 one-hot columns
    (stride 130 keeps 4B alignment); the diag matmul pm_sl^T @ [tm|1|1]
    then yields inter (diagonal) AND pred counts (column 128) in one pass,
    eliminating all separate count matmuls from PE.
  - tgt counts ride free on the ACT one-hot pass via accum_out.

Engine assignment (per chunk):
  DVE : max01/max23/m (bf16 max), pm_c = is_equal(x_c, m) -> bf16 masks,
        memset of the ones-pair columns in tm tiles
  ACT : sq=Square(t-c), tm=Relu(1-sq) -> bf16 one-hot written strided
        (accum_out -> per-partition tgt counts); issues x2/x3 DMAs
  PE  : diag_c += pm_sl^T @ tm_ext_sl  (N=130; diag=inter, col128=pred)
  DMA : sync queue: x0, x1, t; scalar queue: x2, x3

argmax tie semantics: pm_c = (x_c == m) in bf16. Multi-ties inflate counts
by ~0.2% of voxels; measured loss rel-err 1.8e-4.
"""

import sys

sys.path.insert(0, "/opt/trn_rl_repo")

import numpy as np
import ml_dtypes

# ---------------------------------------------------------------------------
# Hardcoded problem geometry
# ---------------------------------------------------------------------------
B = 2
C = 4
N_SP = 128 * 160 * 160        # 3,276,800 voxels per batch
N_CORES = 8
S = N_SP // N_CORES           # 409,600 voxels per core per batch
P = 128
SF = S // P                   # 3200 free elems per partition
# chunk free sizes (multiples of 128); chunk k covers the contiguous flat
# range [o_k*P, (o_k+F_k)*P) reshaped [128, F_k]
CHUNK_F = [1664, 1536]
N_CHUNKS = len(CHUNK_F)
EPS = 1e-08

_CACHE = {}


def _build_bass():
    import concourse.bass as bass
    import concourse.tile as tile
    from concourse import bacc, mybir
    from contextlib import ExitStack

    f32 = mybir.dt.float32
    bf16 = mybir.dt.bfloat16
    u8 = mybir.dt.uint8
    Alu = mybir.AluOpType

    nc = bacc.Bacc()

    n_cols = B * N_CHUNKS * 3
    x = nc.declare_dram_parameter("x", [B, C, S], bf16, isOutput=False)
    t = nc.declare_dram_parameter("t", [B, S], u8, isOutput=False)
    # acc_out[:, col] = tm_c counts (ACT accum), col = (b*N_CHUNKS+k)*3+(c-1)
    acc_d = nc.declare_dram_parameter("acc_out", [P, n_cols], f32, isOutput=True)
    # diag_out[b][:, ci*130 : ci*130+130]: cols 0..127 = pm^T@tm block
    # (trace = inter_c), col 128 = per-column pm sums (sum = pred_cnt_c)
    diag_d = nc.declare_dram_parameter("diag_out", [B, P, 390], f32, isOutput=True)

    with ExitStack() as ctx:
        tc = ctx.enter_context(tile.TileContext(nc))
        const_pool = ctx.enter_context(tc.tile_pool(name="const", bufs=1))
        xpool = ctx.enter_context(tc.tile_pool(name="xp", bufs=3))
        tpool = ctx.enter_context(tc.tile_pool(name="tp", bufs=3))
        mpool = ctx.enter_context(tc.tile_pool(name="mp", bufs=2))
        kpool = ctx.enter_context(tc.tile_pool(name="kp", bufs=2))
        dpool = ctx.enter_context(tc.tile_pool(name="dp", bufs=2))
        pspool = ctx.enter_context(tc.tile_pool(name="ps", bufs=1, space="PSUM"))

        acc = const_pool.tile([P, n_cols], f32)
        # per-partition bias constants for activation
        neg_c = []
        for c in (1, 2, 3):
            bias_t = const_pool.tile([P, 1], f32, tag=f"bias{c}", name=f"bias{c}")
            nc.vector.memset(bias_t[:], -float(c))
            neg_c.append(bias_t)

        for b in range(B):
            ps_diags = [
                pspool.tile(
                    [P, 130], f32, tag=f"diag{b}_{ci}", name=f"ps_diag{b}_{ci}"
                )
                for ci in range(3)
            ]

            for k, F in enumerate(CHUNK_F):
                o = sum(CHUNK_F[:k]) * P
                ns = F // 128
                last = k == N_CHUNKS - 1

                xts = []
                for ci in range(C):
                    xc = xpool.tile([P, F], bf16, tag=f"x{ci}", name=f"x{ci}")
                    xsrc = x[b, ci, o : o + P * F].rearrange("(p f) -> p f", p=P)
                    eng = nc.sync if ci < 2 else nc.scalar
                    eng.dma_start(out=xc[:], in_=xsrc)
                    xts.append(xc)
                tt = tpool.tile([P, F], u8, tag="tt")
                tsrc = t[b, o : o + P * F].rearrange("(p f) -> p f", p=P)
                nc.sync.dma_start(out=tt[:], in_=tsrc)

                m01 = mpool.tile([P, F], bf16, tag="m01")
                nc.vector.tensor_tensor(m01[:], xts[0][:], xts[1][:], op=Alu.max)
                m23 = mpool.tile([P, F], bf16, tag="m23")
                nc.vector.tensor_tensor(m23[:], xts[2][:], xts[3][:], op=Alu.max)
                m = mpool.tile([P, F], bf16, tag="m")
                nc.vector.tensor_tensor(m[:], m01[:], m23[:], op=Alu.max)

                pms = []
                for ci in range(3):
                    pm = kpool.tile([P, F], bf16, tag=f"pm{ci}", name=f"pm{ci}")
                    nc.vector.tensor_tensor(
                        pm[:], xts[ci + 1][:], m[:], op=Alu.is_equal
                    )
                    pms.append(pm)

                tmes = []
                for ci, c in enumerate((1, 2, 3)):
                    sq = kpool.tile([P, F], bf16, tag=f"sq{c}", name=f"sq{c}")
                    nc.scalar.activation(
                        sq[:], tt[:], mybir.ActivationFunctionType.Square,
                        bias=neg_c[ci][:], scale=1.0,
                    )
                    # one-hot blocks interleaved with ones-pairs: layout
                    # [tm(128) | 1 | 1] * ns, stride 130 (4B aligned)
                    tme = dpool.tile([P, ns * 130], bf16, tag=f"tme{c}",
                                     name=f"tme{c}")
                    tv = tme[:, :].rearrange("p (s n) -> p s n", n=130)
                    nc.vector.memset(tv[:, :, 128:130], 1.0)
                    col = (b * N_CHUNKS + k) * 3 + ci
                    nc.scalar.activation(
                        tv[:, :, 0:128],
                        sq[:, :].rearrange("p (s n) -> p s n", n=128),
                        mybir.ActivationFunctionType.Relu,
                        bias=1.0, scale=-1.0,
                        accum_out=acc[:, col : col + 1],
                    )
                    tmes.append(tme)

                # diag_c += pm_sl^T @ [tm_sl | 1 | 1]; diag -> inter,
                # col 128 -> pred counts (col 129 is a duplicate, unused)
                for si in range(ns):
                    sl = slice(si * 128, (si + 1) * 128)
                    sl2 = slice(si * 130, (si + 1) * 130)
                    for ci in range(3):
                        nc.tensor.matmul(
                            ps_diags[ci][:, :],
                            pms[ci][:, sl],
                            tmes[ci][:, sl2],
                            start=(k == 0 and si == 0),
                            stop=(last and si == ns - 1),
                        )

            # drain PSUM -> SBUF (DMA cannot read PSUM), then DMA out
            sb_diag = tpool.tile([P, 390], f32, tag=f"sbd{b}", name=f"sbd{b}")
            for ci in range(3):
                nc.scalar.copy(
                    sb_diag[:, ci * 130 : (ci + 1) * 130], ps_diags[ci][:]
                )
            nc.sync.dma_start(out=diag_d[b, :, :], in_=sb_diag[:])

        nc.sync.dma_start(out=acc_d[:, :], in_=acc[:])

    nc.compile()
    return nc


def _get_nc():
    if "nc" not in _CACHE:
        _CACHE["nc"] = _build_bass()
    return _CACHE["nc"]


def _shard_inputs(input, target):
    inp = np.asarray(input, dtype=np.float32).reshape(B, C, N_SP)
    inp16 = inp.astype(ml_dtypes.bfloat16)
    tgt = np.asarray(target).reshape(B, N_SP)
    in_maps = []
    for r in range(N_CORES):
        xr = np.ascontiguousarray(inp16[:, :, r * S : (r + 1) * S])
        tr = np.ascontiguousarray(tgt[:, r * S : (r + 1) * S].astype(np.uint8))
        in_maps.append({"x": xr, "t": tr})
    return in_maps


def _finish(results):
    """Combine per-core partial counts into the dice loss."""
    inter = np.zeros((B, 3), np.float64)
    pred_cnt = np.zeros((B, 3), np.float64)
    tgt_cnt = np.zeros((B, 3), np.float64)
    for res in results:
        acc = np.asarray(res["acc_out"], np.float64)        # [128, n_cols]
        diag = np.asarray(res["diag_out"], np.float64)      # [B, 128, 390]
        for b in range(B):
            for ci in range(3):
                blk = diag[b][:, ci * 130 : ci * 130 + 128]
                inter[b, ci] += np.trace(blk)
                pred_cnt[b, ci] += diag[b][:, ci * 130 + 128].sum()
                for k in range(N_CHUNKS):
                    tgt_cnt[b, ci] += acc[:, (b * N_CHUNKS + k) * 3 + ci].sum()
    union = pred_cnt + tgt_cnt - inter
    dice = (inter + EPS) / (union + EPS)
    return np.float32(1.0 - dice.mean())


def kernel(input, target):
    from concourse.bass_utils import run_bass_kernel_spmd

    nc = _get_nc()
    in_maps = _shard_inputs(input, target)
    out = run_bass_kernel_spmd(nc, in_maps, core_ids=list(range(N_CORES)))
    return _finish(out.results)


if __name__ == "__main__":
    # Smoke test with random data against a numpy reference.
    rng = np.random.default_rng(0)
    inp = rng.standard_normal((B, C, 128, 160, 160), dtype=np.float32)
    tgt = rng.integers(0, C, size=(B, 128, 160, 160)).astype(np.int32)

    got = kernel(input=inp, target=tgt)

    pred = np.argmax(inp, axis=1).reshape(B, -1)
    tg = tgt.reshape(B, -1)
    dice = np.zeros((B, 3))
    for b in range(B):
        for ci, c in enumerate((1, 2, 3)):
            pm = pred[b] == c
            tm = tg[b] == c
            i = np.sum(pm & tm)
            u = np.sum(pm | tm)
            dice[b, ci] = (i + EPS) / (u + EPS)
    want = np.float32(1.0 - dice.mean())
    print("kernel:", got, "reference:", want, "relerr:", abs(got - want) / abs(want))


# revision 20
# speedup vs baseline: 11.5481x; 1.2965x over previous
"""DiceLoss kernel for Trainium2 (8 NeuronCores, SPMD data-parallel).

Problem: input [2,4,128,160,160] f32 logits, target [2,128,160,160] int
  pred = argmax(input, axis=1); for classes 1..3 compute
  inter_c = |pred==c & tgt==c|, union_c = |pred==c| + |tgt==c| - inter_c
  loss = 1 - mean_{b,c}( (inter+eps)/(union+eps) )

Sharding: flatten spatial dims (N=3,276,800 voxels per batch); each of the
8 cores takes a contiguous 1/8 slice (S=409,600 voxels) of BOTH batches.
Each core computes per-(batch, class) partial counts; the host sums the 8
tiny partial-count tensors and finishes the scalar dice math.

Design (evolved via NTFF profiles: v1 75.9us f32/DVE-bound; v2 64.8us
ACT-1x-bound; ScalarE ACTIVATE measured ~1 elem/cycle regardless of
dtype/layout, so all elementwise work lives on DVE):
  - Host converts logits f32 -> bf16 (loss rel-err 1.8e-4 vs the 2e-2
    gate; bf16 argmax ties are ~0.2% of voxels). Halves HBM traffic and
    doubles DVE tensor_tensor throughput (2x_1P mode).
  - Host converts target labels to bf16 (0..3 exact): DVE
    tensor_scalar(is_equal, c) builds each one-hot plane at 2-4x.
  - Per-(batch,class) target counts are exact host-side bincounts.
  - Chunks cover contiguous flat ranges reshaped [128, F] so every DMA is
    one contiguous block (counts are permutation-invariant).
  - All input DMAs are emitted before any output DMA (engines issue in
    order; an output DMA mid-stream blocks the queue behind it), split
    across both HWDGE queues (sync + scalar), deep prefetch (bufs=5).
  - tm tiles interleave a ones-pair after every 128 one-hot columns
    (stride 130 keeps 4B alignment); the diag matmul pm_sl^T @ [tm|1|1]
    yields inter (diagonal) AND pred counts (column 128) in one pass, so
    PE runs no separate count matmuls.

Engine assignment (per chunk):
  DVE : max01/max23/m (bf16 max), pm_c = is_equal(x_c, m), tm_c one-hot
        (tensor_scalar strided blocks), ones memsets
  ACT : PSUM->SBUF drains only; issues x2/x3 + odd-chunk t DMAs
  PE  : diag_c += pm_sl^T @ tm_ext_sl  (N=130; diag=inter, col128=pred)
  DMA : sync queue: x0, x1 (+even-chunk t); scalar queue: x2, x3

argmax tie semantics: pm_c = (x_c == m) in bf16. Multi-ties inflate counts
by ~0.2% of voxels; measured loss rel-err 1.8e-4.
"""

import sys

sys.path.insert(0, "/opt/trn_rl_repo")

import numpy as np
import ml_dtypes

# ---------------------------------------------------------------------------
# Hardcoded problem geometry
# ---------------------------------------------------------------------------
B = 2
C = 4
N_SP = 128 * 160 * 160        # 3,276,800 voxels per batch
N_CORES = 8
S = N_SP // N_CORES           # 409,600 voxels per core per batch
P = 128
SF = S // P                   # 3200 free elems per partition
# chunk free sizes (multiples of 128); chunk k covers the contiguous flat
# range [o_k*P, (o_k+F_k)*P) reshaped [128, F_k]
CHUNKS_PER_BATCH = [[1664, 1536], [1664, 1536]]
EPS = 1e-08

_CACHE = {}


def _build_bass():
    import concourse.bass as bass
    import concourse.tile as tile
    from concourse import bacc, mybir
    from contextlib import ExitStack

    f32 = mybir.dt.float32
    bf16 = mybir.dt.bfloat16
    Alu = mybir.AluOpType

    nc = bacc.Bacc()

    x = nc.declare_dram_parameter("x", [B, C, S], bf16, isOutput=False)
    t = nc.declare_dram_parameter("t", [B, S], bf16, isOutput=False)
    # diag_out[b][:, ci*130 : ci*130+130]: cols 0..127 = pm^T@tm block
    # (trace = inter_c), col 128 = per-column pm sums (sum = pred_cnt_c)
    diag_d = nc.declare_dram_parameter("diag_out", [B, P, 390], f32, isOutput=True)

    with ExitStack() as ctx:
        tc = ctx.enter_context(tile.TileContext(nc))
        xpool = ctx.enter_context(tc.tile_pool(name="xp", bufs=5))
        tpool = ctx.enter_context(tc.tile_pool(name="tp", bufs=5))
        const_pool = ctx.enter_context(tc.tile_pool(name="const", bufs=1))
        mpool = ctx.enter_context(tc.tile_pool(name="mp", bufs=2))
        kpool = ctx.enter_context(tc.tile_pool(name="kp", bufs=2))
        dpool = ctx.enter_context(tc.tile_pool(name="dp", bufs=2))
        pspool = ctx.enter_context(tc.tile_pool(name="ps", bufs=1, space="PSUM"))

        # Phase A: issue ALL input DMAs up front (an output DMA emitted
        # mid-stream blocks every later input DMA on that queue). Each DMA
        # covers all 128 partitions (partial-partition DMAs engage only a
        # subset of the 16 DMA engines and halve bandwidth — measured).
        tiles = {}
        kk = 0
        for b in range(B):
            chunk_f = CHUNKS_PER_BATCH[b]
            for k, F in enumerate(chunk_f):
                o = sum(chunk_f[:k]) * P
                xts = []
                for ci in range(C):
                    xc = xpool.tile([P, F], bf16, tag=f"x{ci}",
                                    name=f"x{b}{k}{ci}")
                    xsrc = x[b, ci, o : o + P * F].rearrange("(p f) -> p f", p=P)
                    eng = nc.sync if ci < 2 else nc.scalar
                    eng.dma_start(out=xc[:], in_=xsrc)
                    xts.append(xc)
                # t alternates queues to balance per-queue bytes (x is 2+2)
                tt = tpool.tile([P, F], bf16, tag="tt", name=f"tt{b}{k}")
                tsrc = t[b, o : o + P * F].rearrange("(p f) -> p f", p=P)
                teng = nc.sync if kk % 2 == 0 else nc.scalar
                teng.dma_start(out=tt[:], in_=tsrc)
                tiles[(b, k)] = (tt, xts)
                kk += 1

        neg3 = const_pool.tile([P, 1], f32)
        nc.vector.memset(neg3[:], -3.0)

        # Phase B: compute
        all_ps = {}
        for b in range(B):
            ps_diags = [
                pspool.tile(
                    [P, 130], f32, tag=f"diag{b}_{ci}", name=f"ps_diag{b}_{ci}"
                )
                for ci in range(3)
            ]
            all_ps[b] = ps_diags

            chunk_f = CHUNKS_PER_BATCH[b]
            for k, F in enumerate(chunk_f):
                ns = F // 128
                last = k == len(chunk_f) - 1
                tt, xts = tiles[(b, k)]

                m01 = mpool.tile([P, F], bf16, tag="m01")
                nc.vector.tensor_tensor(m01[:], xts[0][:], xts[1][:], op=Alu.max)
                m23 = mpool.tile([P, F], bf16, tag="m23")
                nc.vector.tensor_tensor(m23[:], xts[2][:], xts[3][:], op=Alu.max)
                m = mpool.tile([P, F], bf16, tag="m")
                nc.vector.tensor_tensor(m[:], m01[:], m23[:], op=Alu.max)

                # class 3's one-hot runs on the otherwise-idle ScalarE
                # (Square then strided Relu), concurrent with the DVE max
                # tree; classes 1-2 use DVE tensor_scalar (2-4x mode).
                tme3 = dpool.tile([P, ns * 130], bf16, tag="tme3",
                                  name="tme3")
                tv3 = tme3[:, :].rearrange("p (s n) -> p s n", n=130)
                nc.vector.memset(tv3[:, :, 128:130], 1.0)
                sq3 = dpool.tile([P, F], bf16, tag="sq3", name="sq3")
                nc.scalar.activation(
                    sq3[:], tt[:], mybir.ActivationFunctionType.Square,
                    bias=neg3[:], scale=1.0,
                )
                nc.scalar.activation(
                    tv3[:, :, 0:128],
                    sq3[:, :].rearrange("p (s n) -> p s n", n=128),
                    mybir.ActivationFunctionType.Relu,
                    bias=1.0, scale=-1.0,
                )

                for ci, c in enumerate((1, 2, 3)):
                    pm = kpool.tile([P, F], bf16, tag=f"pm{ci}", name=f"pm{ci}")
                    nc.vector.tensor_tensor(
                        pm[:], xts[ci + 1][:], m[:], op=Alu.is_equal
                    )
                    if c == 3:
                        tme = tme3
                    else:
                        # one-hot blocks interleaved with ones-pairs:
                        # [tm(128) | 1 | 1] * ns, stride 130 (4B aligned)
                        tme = dpool.tile([P, ns * 130], bf16, tag=f"tme{c}",
                                         name=f"tme{c}")
                        tv = tme[:, :].rearrange("p (s n) -> p s n", n=130)
                        nc.vector.memset(tv[:, :, 128:130], 1.0)
                        nc.vector.tensor_scalar(
                            tv[:, :, 0:128],
                            tt[:, :].rearrange("p (s n) -> p s n", n=128),
                            float(c), None, op0=Alu.is_equal,
                        )
                    # diag_c += pm_sl^T @ [tm_sl | 1 | 1]; diag -> inter,
                    # col 128 -> pred counts (col 129 duplicate, unused)
                    for si in range(ns):
                        nc.tensor.matmul(
                            ps_diags[ci][:, :],
                            pm[:, si * 128 : (si + 1) * 128],
                            tme[:, si * 130 : (si + 1) * 130],
                            start=(k == 0 and si == 0),
                            stop=(last and si == ns - 1),
                        )

        # Phase C: drain PSUM -> SBUF (DMA cannot read PSUM), then DMA out
        for b in range(B):
            sb_diag = tpool.tile([P, 390], f32, tag=f"sbd{b}", name=f"sbd{b}")
            for ci in range(3):
                nc.scalar.copy(
                    sb_diag[:, ci * 130 : (ci + 1) * 130], all_ps[b][ci][:]
                )
            nc.sync.dma_start(out=diag_d[b, :, :], in_=sb_diag[:])

    nc.compile()
    return nc


def _get_nc():
    if "nc" not in _CACHE:
        _CACHE["nc"] = _build_bass()
    return _CACHE["nc"]


def _shard_inputs(input, target):
    inp = np.asarray(input, dtype=np.float32).reshape(B, C, N_SP)
    inp16 = inp.astype(ml_dtypes.bfloat16)
    tgt = np.asarray(target).reshape(B, N_SP)
    tgt16 = tgt.astype(ml_dtypes.bfloat16)
    in_maps = []
    for r in range(N_CORES):
        xr = np.ascontiguousarray(inp16[:, :, r * S : (r + 1) * S])
        tr = np.ascontiguousarray(tgt16[:, r * S : (r + 1) * S])
        in_maps.append({"x": xr, "t": tr})
    return in_maps


def _tgt_counts(target):
    """Exact per-(batch, class) target counts, computed host-side."""
    tgt = np.asarray(target).reshape(B, N_SP)
    cnt = np.zeros((B, 3), np.float64)
    for b in range(B):
        bc = np.bincount(tgt[b].astype(np.int64), minlength=C)
        cnt[b] = bc[1:C]
    return cnt


def _finish(results, tgt_cnt):
    """Combine per-core partial counts into the dice loss."""
    inter = np.zeros((B, 3), np.float64)
    pred_cnt = np.zeros((B, 3), np.float64)
    for res in results:
        diag = np.asarray(res["diag_out"], np.float64)      # [B, 128, 390]
        for b in range(B):
            for ci in range(3):
                blk = diag[b][:, ci * 130 : ci * 130 + 128]
                inter[b, ci] += np.trace(blk)
                pred_cnt[b, ci] += diag[b][:, ci * 130 + 128].sum()
    union = pred_cnt + tgt_cnt - inter
    dice = (inter + EPS) / (union + EPS)
    return np.float32(1.0 - dice.mean())


def kernel(input, target):
    from concourse.bass_utils import run_bass_kernel_spmd

    nc = _get_nc()
    in_maps = _shard_inputs(input, target)
    out = run_bass_kernel_spmd(nc, in_maps, core_ids=list(range(N_CORES)))
    return _finish(out.results, _tgt_counts(target))


if __name__ == "__main__":
    # Smoke test with random data against a numpy reference.
    rng = np.random.default_rng(0)
    inp = rng.standard_normal((B, C, 128, 160, 160), dtype=np.float32)
    tgt = rng.integers(0, C, size=(B, 128, 160, 160)).astype(np.int32)

    got = kernel(input=inp, target=tgt)

    pred = np.argmax(inp, axis=1).reshape(B, -1)
    tg = tgt.reshape(B, -1)
    dice = np.zeros((B, 3))
    for b in range(B):
        for ci, c in enumerate((1, 2, 3)):
            pm = pred[b] == c
            tm = tg[b] == c
            i = np.sum(pm & tm)
            u = np.sum(pm | tm)
            dice[b, ci] = (i + EPS) / (u + EPS)
    want = np.float32(1.0 - dice.mean())
    print("kernel:", got, "reference:", want, "relerr:", abs(got - want) / abs(want))


# revision 21
# speedup vs baseline: 11.7612x; 1.0185x over previous
"""DiceLoss kernel for Trainium2 (8 NeuronCores, SPMD data-parallel).

Problem: input [2,4,128,160,160] f32 logits, target [2,128,160,160] int
  pred = argmax(input, axis=1); for classes 1..3 compute
  inter_c = |pred==c & tgt==c|, union_c = |pred==c| + |tgt==c| - inter_c
  loss = 1 - mean_{b,c}( (inter+eps)/(union+eps) )

Sharding: flatten spatial dims (N=3,276,800 voxels per batch); each of the
8 cores takes a contiguous 1/8 slice (S=409,600 voxels) of BOTH batches.
Each core computes per-(batch, class) partial counts; the host sums the 8
tiny partial-count tensors and finishes the scalar dice math.

Design (evolved via NTFF profiles: v1 75.9us f32/DVE-bound; v2 64.8us
ACT-1x-bound; ScalarE ACTIVATE measured ~1 elem/cycle regardless of
dtype/layout, so all elementwise work lives on DVE):
  - Host converts logits f32 -> bf16 (loss rel-err 1.8e-4 vs the 2e-2
    gate; bf16 argmax ties are ~0.2% of voxels). Halves HBM traffic and
    doubles DVE tensor_tensor throughput (2x_1P mode).
  - Host converts target labels to bf16 (0..3 exact): DVE
    tensor_scalar(is_equal, c) builds each one-hot plane at 2-4x.
  - Per-(batch,class) target counts are exact host-side bincounts.
  - Chunks cover contiguous flat ranges reshaped [128, F] so every DMA is
    one contiguous block (counts are permutation-invariant).
  - All input DMAs are emitted before any output DMA (engines issue in
    order; an output DMA mid-stream blocks the queue behind it), split
    across both HWDGE queues (sync + scalar), deep prefetch (bufs=5).
  - tm tiles interleave a ones-pair after every 128 one-hot columns
    (stride 130 keeps 4B alignment); the diag matmul pm_sl^T @ [tm|1|1]
    yields inter (diagonal) AND pred counts (column 128) in one pass, so
    PE runs no separate count matmuls.

Engine assignment (per chunk):
  DVE : max01/max23/m (bf16 max), pm_c = is_equal(x_c, m), tm_c one-hot
        (tensor_scalar strided blocks), ones memsets
  ACT : PSUM->SBUF drains only; issues x2/x3 + odd-chunk t DMAs
  PE  : diag_c += pm_sl^T @ tm_ext_sl  (N=130; diag=inter, col128=pred)
  DMA : sync queue: x0, x1 (+even-chunk t); scalar queue: x2, x3

argmax tie semantics: pm_c = (x_c == m) in bf16. Multi-ties inflate counts
by ~0.2% of voxels; measured loss rel-err 1.8e-4.
"""

import sys

sys.path.insert(0, "/opt/trn_rl_repo")

import numpy as np
import ml_dtypes

# ---------------------------------------------------------------------------
# Hardcoded problem geometry
# ---------------------------------------------------------------------------
B = 2
C = 4
N_SP = 128 * 160 * 160        # 3,276,800 voxels per batch
N_CORES = 8
S = N_SP // N_CORES           # 409,600 voxels per core per batch
P = 128
SF = S // P                   # 3200 free elems per partition
# chunk free sizes (multiples of 128); chunk k covers the contiguous flat
# range [o_k*P, (o_k+F_k)*P) reshaped [128, F_k]
CHUNKS_PER_BATCH = [[1664, 1536], [1664, 1536]]
EPS = 1e-08

_CACHE = {}


def _build_bass():
    import concourse.bass as bass
    import concourse.tile as tile
    from concourse import bacc, mybir
    from contextlib import ExitStack

    f32 = mybir.dt.float32
    bf16 = mybir.dt.bfloat16
    Alu = mybir.AluOpType

    nc = bacc.Bacc()

    x = nc.declare_dram_parameter("x", [B, C, S], bf16, isOutput=False)
    t = nc.declare_dram_parameter("t", [B, S], bf16, isOutput=False)
    # diag_out[b][:, ci*130 : ci*130+130]: cols 0..127 = pm^T@tm block
    # (trace = inter_c), col 128 = per-column pm sums (sum = pred_cnt_c)
    diag_d = nc.declare_dram_parameter("diag_out", [B, P, 390], f32, isOutput=True)

    with ExitStack() as ctx:
        tc = ctx.enter_context(tile.TileContext(nc))
        xpool = ctx.enter_context(tc.tile_pool(name="xp", bufs=5))
        tpool = ctx.enter_context(tc.tile_pool(name="tp", bufs=5))
        const_pool = ctx.enter_context(tc.tile_pool(name="const", bufs=1))
        mpool = ctx.enter_context(tc.tile_pool(name="mp", bufs=2))
        kpool = ctx.enter_context(tc.tile_pool(name="kp", bufs=2))
        dpool = ctx.enter_context(tc.tile_pool(name="dp", bufs=2))
        pspool = ctx.enter_context(tc.tile_pool(name="ps", bufs=1, space="PSUM"))

        # Phase A: issue ALL input DMAs up front (an output DMA emitted
        # mid-stream blocks every later input DMA on that queue). Each DMA
        # covers all 128 partitions (partial-partition DMAs engage only a
        # subset of the 16 DMA engines and halve bandwidth — measured).
        tiles = {}
        kk = 0
        for b in range(B):
            chunk_f = CHUNKS_PER_BATCH[b]
            for k, F in enumerate(chunk_f):
                o = sum(chunk_f[:k]) * P
                xts = []
                for ci in range(C):
                    xc = xpool.tile([P, F], bf16, tag=f"x{ci}",
                                    name=f"x{b}{k}{ci}")
                    xsrc = x[b, ci, o : o + P * F].rearrange("(p f) -> p f", p=P)
                    eng = nc.sync if ci < 2 else nc.scalar
                    eng.dma_start(out=xc[:], in_=xsrc)
                    xts.append(xc)
                # t alternates queues to balance per-queue bytes (x is 2+2)
                tt = tpool.tile([P, F], bf16, tag="tt", name=f"tt{b}{k}")
                tsrc = t[b, o : o + P * F].rearrange("(p f) -> p f", p=P)
                teng = nc.sync if kk % 2 == 0 else nc.scalar
                teng.dma_start(out=tt[:], in_=tsrc)
                tiles[(b, k)] = (tt, xts)
                kk += 1

        neg3 = const_pool.tile([P, 1], f32)
        nc.vector.memset(neg3[:], -3.0)

        # Phase B: compute
        all_ps = {}
        for b in range(B):
            ps_diags = [
                pspool.tile(
                    [P, 130], f32, tag=f"diag{b}_{ci}", name=f"ps_diag{b}_{ci}"
                )
                for ci in range(3)
            ]
            all_ps[b] = ps_diags

            chunk_f = CHUNKS_PER_BATCH[b]
            for k, F in enumerate(chunk_f):
                ns = F // 128
                last = k == len(chunk_f) - 1
                tt, xts = tiles[(b, k)]

                m01 = mpool.tile([P, F], bf16, tag="m01")
                nc.vector.tensor_tensor(m01[:], xts[0][:], xts[1][:], op=Alu.max)
                m23 = mpool.tile([P, F], bf16, tag="m23")
                nc.vector.tensor_tensor(m23[:], xts[2][:], xts[3][:], op=Alu.max)
                m = mpool.tile([P, F], bf16, tag="m")
                nc.vector.tensor_tensor(m[:], m01[:], m23[:], op=Alu.max)

                # class 3's one-hot runs on the otherwise-idle ScalarE
                # (Square then strided Relu), concurrent with the DVE max
                # tree; classes 1-2 use DVE tensor_scalar (2-4x mode).
                tme3 = dpool.tile([P, ns * 130], bf16, tag="tme3",
                                  name="tme3")
                tv3 = tme3[:, :].rearrange("p (s n) -> p s n", n=130)
                nc.vector.memset(tv3[:, :, 128:130], 1.0)
                sq3 = dpool.tile([P, F], bf16, tag="sq3", name="sq3")
                nc.scalar.activation(
                    sq3[:], tt[:], mybir.ActivationFunctionType.Square,
                    bias=neg3[:], scale=1.0,
                )
                nc.scalar.activation(
                    tv3[:, :, 0:128],
                    sq3[:, :].rearrange("p (s n) -> p s n", n=128),
                    mybir.ActivationFunctionType.Relu,
                    bias=1.0, scale=-1.0,
                )

                # class 3 first: its one-hot comes from ScalarE (which runs
                # early, concurrent with the max tree), so the chunk's last
                # matmuls are gated by DVE's own fast tensor_scalar classes
                for ci, c in ((2, 3), (0, 1), (1, 2)):
                    pm = kpool.tile([P, F], bf16, tag=f"pm{ci}", name=f"pm{ci}")
                    nc.vector.tensor_tensor(
                        pm[:], xts[ci + 1][:], m[:], op=Alu.is_equal
                    )
                    if c == 3:
                        tme = tme3
                    else:
                        # one-hot blocks interleaved with ones-pairs:
                        # [tm(128) | 1 | 1] * ns, stride 130 (4B aligned)
                        tme = dpool.tile([P, ns * 130], bf16, tag=f"tme{c}",
                                         name=f"tme{c}")
                        tv = tme[:, :].rearrange("p (s n) -> p s n", n=130)
                        nc.vector.memset(tv[:, :, 128:130], 1.0)
                        nc.vector.tensor_scalar(
                            tv[:, :, 0:128],
                            tt[:, :].rearrange("p (s n) -> p s n", n=128),
                            float(c), None, op0=Alu.is_equal,
                        )
                    # diag_c += pm_sl^T @ [tm_sl | 1 | 1]; diag -> inter,
                    # col 128 -> pred counts (col 129 duplicate, unused)
                    for si in range(ns):
                        nc.tensor.matmul(
                            ps_diags[ci][:, :],
                            pm[:, si * 128 : (si + 1) * 128],
                            tme[:, si * 130 : (si + 1) * 130],
                            start=(k == 0 and si == 0),
                            stop=(last and si == ns - 1),
                        )

        # Phase C: drain PSUM -> SBUF (DMA cannot read PSUM), then DMA out
        for b in range(B):
            sb_diag = tpool.tile([P, 390], f32, tag=f"sbd{b}", name=f"sbd{b}")
            for ci in range(3):
                nc.scalar.copy(
                    sb_diag[:, ci * 130 : (ci + 1) * 130], all_ps[b][ci][:]
                )
            nc.sync.dma_start(out=diag_d[b, :, :], in_=sb_diag[:])

    nc.compile()
    return nc


def _get_nc():
    if "nc" not in _CACHE:
        _CACHE["nc"] = _build_bass()
    return _CACHE["nc"]


def _shard_inputs(input, target):
    inp = np.asarray(input, dtype=np.float32).reshape(B, C, N_SP)
    inp16 = inp.astype(ml_dtypes.bfloat16)
    tgt = np.asarray(target).reshape(B, N_SP)
    tgt16 = tgt.astype(ml_dtypes.bfloat16)
    in_maps = []
    for r in range(N_CORES):
        xr = np.ascontiguousarray(inp16[:, :, r * S : (r + 1) * S])
        tr = np.ascontiguousarray(tgt16[:, r * S : (r + 1) * S])
        in_maps.append({"x": xr, "t": tr})
    return in_maps


def _tgt_counts(target):
    """Exact per-(batch, class) target counts, computed host-side."""
    tgt = np.asarray(target).reshape(B, N_SP)
    cnt = np.zeros((B, 3), np.float64)
    for b in range(B):
        bc = np.bincount(tgt[b].astype(np.int64), minlength=C)
        cnt[b] = bc[1:C]
    return cnt


def _finish(results, tgt_cnt):
    """Combine per-core partial counts into the dice loss."""
    inter = np.zeros((B, 3), np.float64)
    pred_cnt = np.zeros((B, 3), np.float64)
    for res in results:
        diag = np.asarray(res["diag_out"], np.float64)      # [B, 128, 390]
        for b in range(B):
            for ci in range(3):
                blk = diag[b][:, ci * 130 : ci * 130 + 128]
                inter[b, ci] += np.trace(blk)
                pred_cnt[b, ci] += diag[b][:, ci * 130 + 128].sum()
    union = pred_cnt + tgt_cnt - inter
    dice = (inter + EPS) / (union + EPS)
    return np.float32(1.0 - dice.mean())


def kernel(input, target):
    from concourse.bass_utils import run_bass_kernel_spmd

    nc = _get_nc()
    in_maps = _shard_inputs(input, target)
    out = run_bass_kernel_spmd(nc, in_maps, core_ids=list(range(N_CORES)))
    return _finish(out.results, _tgt_counts(target))


if __name__ == "__main__":
    # Smoke test with random data against a numpy reference.
    rng = np.random.default_rng(0)
    inp = rng.standard_normal((B, C, 128, 160, 160), dtype=np.float32)
    tgt = rng.integers(0, C, size=(B, 128, 160, 160)).astype(np.int32)

    got = kernel(input=inp, target=tgt)

    pred = np.argmax(inp, axis=1).reshape(B, -1)
    tg = tgt.reshape(B, -1)
    dice = np.zeros((B, 3))
    for b in range(B):
        for ci, c in enumerate((1, 2, 3)):
            pm = pred[b] == c
            tm = tg[b] == c
            i = np.sum(pm & tm)
            u = np.sum(pm | tm)
            dice[b, ci] = (i + EPS) / (u + EPS)
    want = np.float32(1.0 - dice.mean())
    print("kernel:", got, "reference:", want, "relerr:", abs(got - want) / abs(want))
